# revision 1
# baseline (speedup 1.0000x reference)
"""Trainium2 Bass/Tile kernel for a 5-block 1D-CNN stack
(ChannelNorm -> ReLU -> Conv1d(k=4..8) -> sigmoid gate -> re-pad).

Data-parallel over batch: 32 samples -> 4 per NeuronCore x 8 cores.

Per-core layout strategy:
  * activations kept in layout B: [S(partitions), C(free)] so the channel
    norm (reduce over C) is a free-dim bn_stats and the per-position
    norm / gate scalars are per-partition ACT scale/bias operands.
  * conv runs as PE matmuls contracting over C_in, which needs layout A
    [C(partitions), S(free)]; PE transposes (matmul w/ identity) convert
    B->A for the conv input and A->B for the conv output.
  * the sigmoid gate multiplier g = 1+score is per-position, constant
    over channels, so for blocks 1..4 it is folded algebraically into the
    next block's norm coefficients (scale = g*rstd', bias = -mean*g*rstd',
    with var scaled by g^2 before adding eps -- exact), costing zero full
    passes.  Only the final block materializes the gated output.

Matmuls (conv + gate + transposes) run as float32r: full PE rate at
moving-dim >= 256 with near-fp32 accuracy, fp32 PSUM accumulation.
"""

import numpy as np

import concourse.bass as bass
import concourse.mybir as mybir
import concourse.tile as tile
from concourse import bacc
from concourse.bass_utils import run_bass_kernel_spmd
from concourse.masks import make_identity

B, S, C = 32, 1024, 256
NCORES = 8
BL = B // NCORES  # batch per core
KS = [4, 5, 6, 7, 8]
PADS = {4: 1, 5: 2, 6: 2, 7: 3, 8: 3}
LS = {k: S + 2 * PADS[k] - k + 1 for k in KS}  # conv output lengths
HALO_L, HALO_R = 3, 5
XW = HALO_L + S + HALO_R  # 1032: layout-A activation width incl. zero halo
EPS = 1e-5
UNB = float(C) / float(C - 1)  # unbiased-variance correction (ddof=1)
NT = S // 128  # 8 s-tiles of 128 per sample
F32 = mybir.dt.float32
F32R = mybir.dt.float32r

_CACHED_NC = None


def _mm(nc, out, lhsT, rhs, start, stop):
    nc.tensor.matmul(out, lhsT, rhs, start=start, stop=stop)


def _build_block(nc, tc, pools, blk, k, xb_b_tiles, g_tile, w_sb, fc_sb,
                 identity, eps_sb, zr, out_dram):
    """Emit one conv block for all BL local batches.

    xb_b_tiles: list of BL SBUF tiles [128, NT, 256] (layout B, pre-gate).
    g_tile: None (block 0) or [128, BL, NT] gate multiplier from prev block.
    Returns (next_xb_tiles, next_g_tile). For the last block, writes output
    DMAs and returns (None, None).
    """
    p = PADS[k]
    L = LS[k]
    last = (blk == len(KS) - 1)
    sm, xnb_p, xna_p, ha_p, psc, pst, pshb, psy, xb_pool, out_p = pools

    g_next = sm.tile([128, BL, NT], F32, tag="g")
    next_xb = None if last else []

    for b in range(BL):
        xb_b = xb_b_tiles[b]

        # ---- channel-norm stats over C (free dim) ----
        bn6 = sm.tile([128, NT, 6], F32, tag="bn6")
        for st in range(NT):
            nc.vector.bn_stats(out=bn6[:, st, :], in_=xb_b[:, st, :])
        mv = sm.tile([128, NT, 2], F32, tag="mv")
        for st in range(NT):
            nc.vector.bn_aggr(out=mv[:, st, :], in_=bn6[:, st, :])
        mean = mv[:, :, 0:1].rearrange("p t o -> p (t o)")  # [128, NT]
        var = mv[:, :, 1:2].rearrange("p t o -> p (t o)")

        # ---- norm coefficients (gate of prev block folded in) ----
        rt = sm.tile([128, NT], F32, tag="rt")
        g2v = sm.tile([128, NT], F32, tag="g2v")
        if g_tile is None:
            # single DVE reader collapses the 8 bn_aggr deps into one
            # same-engine chain (walrus caps sync waits per instruction)
            nc.vector.tensor_copy(g2v, var)
        else:
            g_b = g_tile[:, b, :]  # [128, NT]
            nc.vector.tensor_mul(g2v, g_b, g_b)
            nc.vector.tensor_mul(g2v, g2v, var)
        # rt = sqrt(g^2 * var * UNB + eps)
        nc.scalar.activation(out=rt, in_=g2v,
                             func=mybir.ActivationFunctionType.Sqrt,
                             bias=eps_sb, scale=UNB)
        rr = sm.tile([128, NT], F32, tag="rr")
        nc.vector.reciprocal(rr, rt)
        scale_c = sm.tile([128, NT], F32, tag="scale_c")
        if g_tile is None:
            nc.vector.tensor_copy(scale_c, rr)
        else:
            nc.vector.tensor_mul(scale_c, rr, g_tile[:, b, :])
        bias_c = sm.tile([128, NT], F32, tag="bias_c")
        # bias = -(mean * scale)
        nc.vector.scalar_tensor_tensor(out=bias_c, in0=mean, scalar=-1.0,
                                       in1=scale_c,
                                       op0=mybir.AluOpType.mult,
                                       op1=mybir.AluOpType.mult)

        # ---- fused normalize + relu (layout B), then transpose to A ----
        xna_b = xna_p.tile([128, 2, XW], F32R, tag="xnA")  # [ci, s+halo]
        # f32r halo zeros via copy (memset on f32r fails the ISA check)
        nc.vector.tensor_copy(out=xna_b[:, :, 0:HALO_L],
                              in_=zr[:, :, 0:HALO_L])
        nc.vector.tensor_copy(out=xna_b[:, :, HALO_L + S:XW],
                              in_=zr[:, :, 0:HALO_R])
        for tg in range(NT // 4):  # groups of 4 s-tiles -> one psum bank
            xnb_ts = []
            for st in range(4 * tg, 4 * tg + 4):
                xnb_t = xnb_p.tile([128, C], F32R, tag="xnB")
                nc.scalar.activation(out=xnb_t, in_=xb_b[:, st, :],
                                     func=mybir.ActivationFunctionType.Relu,
                                     scale=scale_c[:, st:st + 1],
                                     bias=bias_c[:, st:st + 1])
                xnb_ts.append(xnb_t)
            for ci in range(2):
                ps = pst.tile([128, 512], F32R, tag="pst")
                for j, xnb_t in enumerate(xnb_ts):
                    nc.tensor.transpose(ps[:, j * 128:(j + 1) * 128],
                                        xnb_t[:, ci * 128:(ci + 1) * 128],
                                        identity)
                nc.vector.tensor_copy(
                    out=xna_b[:, ci, HALO_L + tg * 512:HALO_L + tg * 512 + 512],
                    in_=ps)

        # ---- conv as matmuls (contract over ci x dk), layout A out ----
        ha_b = ha_p.tile([128, 2, S], F32R, tag="hA")  # [co_chunk, s]
        for t in range(2):
            for co in range(2):
                pc = psc.tile([128, 512], F32, tag="psc")
                idx = 0
                for ci in range(2):
                    for dk in range(k):
                        base = HALO_L - p + t * 512 + dk
                        _mm(nc, pc,
                            w_sb[:, ci, co, dk, :],
                            xna_b[:, ci, base:base + 512],
                            start=(idx == 0), stop=(idx == 2 * k - 1))
                        idx += 1
                nc.scalar.copy(out=ha_b[:, co, t * 512:(t + 1) * 512],
                               in_=pc)
        if L < S:  # zero the re-pad tail (fp32r zeros via copy)
            for co in range(2):
                nc.vector.tensor_copy(out=ha_b[:, co, L:S],
                                      in_=zr[:, 0, 0:S - L])

        # ---- gate: y^T[s, cls] = h^T @ fc, s on partitions directly ----
        ya2 = sm.tile([128, NT, 2], F32, tag="ya2")
        for st in range(NT):
            py = psy.tile([128, 2], F32, tag="psy")
            for co in range(2):
                _mm(nc, py, ha_b[:, co, st * 128:(st + 1) * 128],
                    fc_sb[:, co, :], start=(co == 0), stop=(co == 1))
            nc.vector.tensor_copy(out=ya2[:, st, :], in_=py)
        ya = ya2[:, :, 0:1].rearrange("p t o -> p (t o)")  # [128, NT]
        yb = ya2[:, :, 1:2].rearrange("p t o -> p (t o)")
        smax = sm.tile([128, NT], F32, tag="smax")
        nc.vector.tensor_max(smax, ya, yb)
        ssum = sm.tile([128, NT], F32, tag="ssum")
        nc.vector.tensor_add(ssum, ya, yb)
        pre = sm.tile([128, NT], F32, tag="pre")
        # pre = 0.2*max + (ya+yb);  score = sigmoid(0.5*pre)
        nc.vector.scalar_tensor_tensor(out=pre, in0=smax, scalar=0.2,
                                       in1=ssum,
                                       op0=mybir.AluOpType.mult,
                                       op1=mybir.AluOpType.add)
        sc = sm.tile([128, NT], F32, tag="sc")
        nc.scalar.activation(out=sc, in_=pre,
                             func=mybir.ActivationFunctionType.Sigmoid,
                             scale=0.5)
        nc.vector.tensor_scalar_add(g_next[:, b, :], sc, 1.0)

        # ---- transpose conv output back to layout B ----
        if last:
            dst_t = out_p.tile([128, NT, C], F32, tag="osb")
        else:
            dst_t = xb_pool.tile([128, NT, C], F32, tag="xB")
            next_xb.append(dst_t)
        for st in range(NT):
            ph = pshb.tile([128, C], F32R, tag="pshb")
            for co in range(2):
                nc.tensor.transpose(ph[:, co * 128:(co + 1) * 128],
                                    ha_b[:, co, st * 128:(st + 1) * 128],
                                    identity)
            if last:
                nc.vector.tensor_scalar_mul(out=dst_t[:, st, :],
                                            in0=ph,
                                            scalar1=g_next[:, b, st:st + 1])
            else:
                nc.vector.tensor_copy(out=dst_t[:, st, :], in_=ph)

        if last:
            dst = out_dram[b].rearrange("(t p) c -> p t c", p=128)
            nc.sync.dma_start(out=dst, in_=dst_t)

    return next_xb, g_next


def _build():
    nc = bacc.Bacc("TRN2", target_bir_lowering=False, debug=False,
                   num_devices=NCORES)
    x_in = nc.dram_tensor("x", [BL, S, C], F32, kind="ExternalInput").ap()
    w_in = {k: nc.dram_tensor(f"w{k}", [128, 2, 2, k, 128], F32R,
                              kind="ExternalInput").ap() for k in KS}
    fc_in = nc.dram_tensor("fc", [128, 2, 2], F32R,
                           kind="ExternalInput").ap()
    out_dram = nc.dram_tensor("out", [BL, S, C], F32,
                              kind="ExternalOutput").ap()

    from contextlib import ExitStack
    with tile.TileContext(nc) as tc, ExitStack() as ctx:
        consts = ctx.enter_context(tc.tile_pool(name="consts", bufs=1))
        wpool = ctx.enter_context(tc.tile_pool(name="wpool", bufs=2))
        xb_pool = ctx.enter_context(tc.tile_pool(name="xb", bufs=6))
        xnb_p = ctx.enter_context(tc.tile_pool(name="xnb", bufs=6))
        xna_p = ctx.enter_context(tc.tile_pool(name="xna", bufs=4))
        ha_p = ctx.enter_context(tc.tile_pool(name="ha", bufs=4))
        sm = ctx.enter_context(tc.tile_pool(name="small", bufs=8))
        out_p = ctx.enter_context(tc.tile_pool(name="outp", bufs=2))
        psc = ctx.enter_context(tc.tile_pool(name="psc", bufs=2, space="PSUM"))
        pst = ctx.enter_context(tc.tile_pool(name="pst", bufs=2, space="PSUM"))
        pshb = ctx.enter_context(tc.tile_pool(name="pshb", bufs=2,
                                              space="PSUM"))
        psy = ctx.enter_context(tc.tile_pool(name="psy", bufs=2, space="PSUM"))

        identity_f = consts.tile([128, 128], F32)
        make_identity(nc, identity_f)
        identity = consts.tile([128, 128], F32R)
        nc.vector.tensor_copy(out=identity, in_=identity_f)
        zr_f = consts.tile([128, 2, 8], F32)
        nc.vector.memset(zr_f, 0.0)
        zr = consts.tile([128, 2, 8], F32R)
        nc.vector.tensor_copy(out=zr, in_=zr_f)
        fc_sb = consts.tile([128, 2, 2], F32R)
        nc.sync.dma_start(out=fc_sb, in_=fc_in)
        eps_sb = consts.tile([128, 1], F32)
        nc.vector.memset(eps_sb, EPS)

        # initial load: [BL, S, C] -> per-batch layout-B tiles
        xb_tiles = []
        for b in range(BL):
            t = xb_pool.tile([128, NT, C], F32, tag="xB")
            nc.sync.dma_start(out=t,
                              in_=x_in[b].rearrange("(t p) c -> p t c", p=128))
            xb_tiles.append(t)

        pools = (sm, xnb_p, xna_p, ha_p, psc, pst, pshb, psy,
                 xb_pool, out_p)
        g_tile = None
        for blk, k in enumerate(KS):
            w_sb = wpool.tile([128, 2, 2, k, 128], F32R, tag="w")
            nc.sync.dma_start(out=w_sb, in_=w_in[k])
            xb_tiles, g_tile = _build_block(
                nc, tc, pools, blk, k, xb_tiles, g_tile, w_sb, fc_sb,
                identity, eps_sb, zr, out_dram)

    nc.compile()
    return nc


def _get_nc():
    global _CACHED_NC
    if _CACHED_NC is None:
        _CACHED_NC = _build()
    return _CACHED_NC


def _prep_weights(inputs):
    """Host-side packing of conv / fc weights into the DRAM layouts."""
    arrs = {}
    for k in KS:
        W = np.asarray(inputs[f"W{k}"], np.float32)  # [co, ci, k]
        Wt = W.transpose(1, 0, 2)                    # [ci, co, k]
        Wt = Wt.reshape(2, 128, 2, 128, k)           # [ci_ch, ci_in, co_ch, co_in, k]
        Wt = Wt.transpose(1, 0, 2, 4, 3)             # [ci_in, ci_ch, co_ch, k, co_in]
        arrs[f"w{k}"] = np.ascontiguousarray(Wt, np.float32)
    fc = np.asarray(inputs["fc_w"], np.float32)[:, :, 0]  # [2, co=256]
    fcT = fc.T.reshape(2, 128, 2).transpose(1, 0, 2)      # [co_in, co_ch, cls]
    arrs["fc"] = np.ascontiguousarray(fcT, np.float32)
    return arrs


def _apply_cn_affine(inputs):
    """The kernel folds ChannelNorm's (w, b) away assuming w==1, b==0
    (true for this model's initialization). Verify on host."""
    for k in KS:
        w = np.asarray(inputs[f"cn{k}_w"], np.float32)
        bb = np.asarray(inputs[f"cn{k}_b"], np.float32)
        if not (np.allclose(w, 1.0, atol=1e-6) and
                np.allclose(bb, 0.0, atol=1e-6)):
            raise NotImplementedError(
                "kernel assumes channel-norm weight==1, bias==0")


def kernel(run_opts=None, **inputs):
    _apply_cn_affine(inputs)
    nc = _get_nc()
    warrs = _prep_weights(inputs)
    x = np.ascontiguousarray(np.asarray(inputs["inputs"], np.float32))
    in_maps = []
    for c in range(NCORES):
        m = {"x": np.ascontiguousarray(x[c * BL:(c + 1) * BL])}
        m.update(warrs)
        in_maps.append(m)
    res = run_bass_kernel_spmd(nc, in_maps, core_ids=list(range(NCORES)),
                               **(run_opts or {}))
    out = np.concatenate([r["out"] for r in res.results], axis=0)
    if run_opts:
        return out, res
    return out



# revision 2
# speedup vs baseline: 1.0860x; 1.0860x over previous
"""Trainium2 Bass/Tile kernel for a 5-block 1D-CNN stack
(ChannelNorm -> ReLU -> Conv1d(k=4..8) -> sigmoid gate -> re-pad).

Data-parallel over batch: 32 samples -> 4 per NeuronCore x 8 cores.

Per-core layout strategy (v2):
  * activations in layout B [S(part), C(free)] for the channel norm
    (free-dim bn_stats) and the fused normalize+relu (per-partition
    ACT scale/bias); conv runs in layout A [C(part), S(free)] as PE
    matmuls contracting over C_in.
  * ALL layout conversions use the DMA XBAR transpose (16-bit only):
    one descriptor turns [128, T*128] into [128, T, 128] blocks with
    out[p,t,q] = in[q, t*128+p]. This moves the transposes off the PE
    (which is the roofline engine: the conv alone is ~205us of PE) and
    kills every PSUM->SBUF transpose copy.
  * conv + gate matmuls run in bf16 (1 cycle/row, fp32 PSUM accum).
  * the sigmoid gate multiplier g = 1+score is folded into the next
    block's norm coefficients (exact), as in v1; only the last block
    materializes the gated output.
  * scalar-engine funcs are restricted to {Relu, Copy, Ln, Exp} which
    share one activation table set: rstd = exp(-0.5*ln(var')) and
    sigmoid(z) = 1/(1+exp(-z)) (DVE reciprocal), so no per-iteration
    LoadActFuncSet swaps.
"""

import numpy as np
import ml_dtypes

import concourse.bass as bass
import concourse.mybir as mybir
import concourse.tile as tile
from concourse import bacc
from concourse.bass_utils import run_bass_kernel_spmd

B, S, C = 32, 1024, 256
NCORES = 8
BL = B // NCORES  # batch per core
KS = [4, 5, 6, 7, 8]
PADS = {4: 1, 5: 2, 6: 2, 7: 3, 8: 3}
LS = {k: S + 2 * PADS[k] - k + 1 for k in KS}  # conv output lengths
EPS = 1e-5
UNB = float(C) / float(C - 1)  # unbiased-variance correction (ddof=1)
NT = S // 128  # 8 s-tiles of 128 per sample
F32 = mybir.dt.float32
BF16 = mybir.dt.bfloat16
AF = mybir.ActivationFunctionType

_CACHED_NC = None


def _build_block(nc, tc, pools, blk, k, xb_b_tiles, g_tile, w_sb, fc_sb,
                 eps_sb, out_dram):
    """Emit one conv block for all BL local batches.

    xb_b_tiles: list of BL SBUF tiles [128, NT, 256] (layout B, pre-gate).
    g_tile: None (block 0) or [128, BL, NT] gate multiplier from prev block.
    Returns (next_xb_tiles, next_g_tile). For the last block, writes output
    DMAs and returns (None, None).
    """
    p = PADS[k]
    L = LS[k]
    last = (blk == len(KS) - 1)
    sm, xnb_p, xna_p, ha_p, psc, psy, xb_pool, hb_pool, out_p = pools

    g_next = sm.tile([128, BL, NT], F32, tag="g")
    next_xb = None if last else []

    for b in range(BL):
        xb_b = xb_b_tiles[b]

        # ---- channel-norm stats over C (free dim) ----
        bn6 = sm.tile([128, NT, 6], F32, tag="bn6")
        for st in range(NT):
            nc.vector.bn_stats(out=bn6[:, st, :], in_=xb_b[:, st, :])
        mv = sm.tile([128, NT, 2], F32, tag="mv")
        for st in range(NT):
            nc.vector.bn_aggr(out=mv[:, st, :], in_=bn6[:, st, :])
        mean = mv[:, :, 0:1].rearrange("p t o -> p (t o)")  # [128, NT]
        var = mv[:, :, 1:2].rearrange("p t o -> p (t o)")

        # ---- norm coefficients (gate of prev block folded in) ----
        g2v = sm.tile([128, NT], F32, tag="g2v")
        if g_tile is None:
            # single DVE reader collapses the 8 bn_aggr deps into one
            # same-engine chain (walrus caps sync waits per instruction)
            nc.vector.tensor_copy(g2v, var)
        else:
            g_b = g_tile[:, b, :]  # [128, NT]
            nc.vector.tensor_mul(g2v, g_b, g_b)
            nc.vector.tensor_mul(g2v, g2v, var)
        # rr = 1/sqrt(g^2 * var * UNB + eps) = exp(-0.5 * ln(...))
        # (keeps the scalar engine inside one act-table set: {Ln,Exp,Relu,Copy})
        lnv = sm.tile([128, NT], F32, tag="lnv")
        nc.scalar.activation(out=lnv, in_=g2v, func=AF.Ln,
                             bias=eps_sb, scale=UNB)
        rr = sm.tile([128, NT], F32, tag="rr")
        nc.scalar.activation(out=rr, in_=lnv, func=AF.Exp, scale=-0.5)
        scale_c = sm.tile([128, NT], F32, tag="scale_c")
        if g_tile is None:
            nc.vector.tensor_copy(scale_c, rr)
        else:
            nc.vector.tensor_mul(scale_c, rr, g_tile[:, b, :])
        bias_c = sm.tile([128, NT], F32, tag="bias_c")
        # bias = -(mean * scale)
        nc.vector.scalar_tensor_tensor(out=bias_c, in0=mean, scalar=-1.0,
                                       in1=scale_c,
                                       op0=mybir.AluOpType.mult,
                                       op1=mybir.AluOpType.mult)

        # ---- fused normalize + relu (layout B, bf16 out), XBAR to A ----
        xna_b = xna_p.tile([128, 2, S], BF16, tag="xnA")  # [ci_ch, s]
        for st in range(NT):
            xnb_t = xnb_p.tile([128, C], BF16, tag="xnB")
            nc.scalar.activation(out=xnb_t, in_=xb_b[:, st, :],
                                 func=AF.Relu,
                                 scale=scale_c[:, st:st + 1],
                                 bias=bias_c[:, st:st + 1])
            # out[ci, ch, s] = xnb[s, ch*128+ci]
            nc.sync.dma_start(out=xna_b[:, :, st * 128:(st + 1) * 128],
                              in_=xnb_t, transpose=True)

        # ---- conv as matmuls (contract ci, accumulate taps), layout A ----
        ha_b = ha_p.tile([128, 2, S], BF16, tag="hA")  # [co_ch, s]
        for t in range(2):
            for co in range(2):
                pc = psc.tile([128, 512], F32, tag="psc")
                # tap order: (ci=0, dk=p) first -- full 512 width so
                # start=True initializes the whole psum tile.
                taps = [(0, p)] + [(ci, dk) for ci in range(2)
                                   for dk in range(k) if (ci, dk) != (0, p)]
                for idx, (ci, dk) in enumerate(taps):
                    lo = t * 512 + dk - p  # src col of psum col 0
                    a = max(0, -lo)
                    bb = 512 - max(0, lo + 512 - S)
                    nc.tensor.matmul(pc[:, a:bb],
                                     w_sb[:, ci, co, dk, :],
                                     xna_b[:, ci, lo + a:lo + bb],
                                     start=(idx == 0),
                                     stop=(idx == len(taps) - 1))
                nc.scalar.copy(out=ha_b[:, co, t * 512:(t + 1) * 512],
                               in_=pc)
        if L < S:  # zero the re-pad tail
            for co in range(2):
                nc.vector.memset(ha_b[:, co, L:S], 0.0)

        # ---- gate: y^T[s, cls] = h^T @ fc, s on partitions directly ----
        ya2 = sm.tile([128, NT, 2], F32, tag="ya2")
        for st in range(NT):
            py = psy.tile([128, 2], F32, tag="psy")
            for co in range(2):
                nc.tensor.matmul(py, ha_b[:, co, st * 128:(st + 1) * 128],
                                 fc_sb[:, co, :],
                                 start=(co == 0), stop=(co == 1))
            nc.vector.tensor_copy(out=ya2[:, st, :], in_=py)
        ya = ya2[:, :, 0:1].rearrange("p t o -> p (t o)")  # [128, NT]
        yb = ya2[:, :, 1:2].rearrange("p t o -> p (t o)")
        smax = sm.tile([128, NT], F32, tag="smax")
        nc.vector.tensor_max(smax, ya, yb)
        ssum = sm.tile([128, NT], F32, tag="ssum")
        nc.vector.tensor_add(ssum, ya, yb)
        pre = sm.tile([128, NT], F32, tag="pre")
        # pre = 0.2*max + (ya+yb);  score = sigmoid(0.5*pre)
        nc.vector.scalar_tensor_tensor(out=pre, in0=smax, scalar=0.2,
                                       in1=ssum,
                                       op0=mybir.AluOpType.mult,
                                       op1=mybir.AluOpType.add)
        # sigmoid via exp + reciprocal (stays in the one act-table set)
        ex = sm.tile([128, NT], F32, tag="ex")
        nc.scalar.activation(out=ex, in_=pre, func=AF.Exp, scale=-0.5)
        den = sm.tile([128, NT], F32, tag="den")
        nc.vector.tensor_scalar_add(den, ex, 1.0)
        sig = sm.tile([128, NT], F32, tag="sig")
        nc.vector.reciprocal(sig, den)
        nc.vector.tensor_scalar_add(g_next[:, b, :], sig, 1.0)

        # ---- XBAR transpose conv output back to layout B ----
        if last:
            hb_t = hb_pool.tile([128, NT, C], BF16, tag="hb")
            for co in range(2):
                nc.sync.dma_start(out=hb_t[:, :, co * 128:(co + 1) * 128],
                                  in_=ha_b[:, co, :], transpose=True)
            dst_t = out_p.tile([128, NT, C], F32, tag="osb")
            for st in range(NT):
                nc.vector.tensor_scalar_mul(out=dst_t[:, st, :],
                                            in0=hb_t[:, st, :],
                                            scalar1=g_next[:, b, st:st + 1])
            dst = out_dram[b].rearrange("(t p) c -> p t c", p=128)
            nc.sync.dma_start(out=dst, in_=dst_t)
        else:
            dst_t = xb_pool.tile([128, NT, C], BF16, tag="xB")
            for co in range(2):
                nc.sync.dma_start(out=dst_t[:, :, co * 128:(co + 1) * 128],
                                  in_=ha_b[:, co, :], transpose=True)
            next_xb.append(dst_t)

    return next_xb, g_next


def _build():
    nc = bacc.Bacc("TRN2", target_bir_lowering=False, debug=False,
                   num_devices=NCORES)
    x_in = nc.dram_tensor("x", [BL, S, C], F32, kind="ExternalInput").ap()
    w_in = {k: nc.dram_tensor(f"w{k}", [128, 2, 2, k, 128], BF16,
                              kind="ExternalInput").ap() for k in KS}
    fc_in = nc.dram_tensor("fc", [128, 2, 2], BF16,
                           kind="ExternalInput").ap()
    out_dram = nc.dram_tensor("out", [BL, S, C], F32,
                              kind="ExternalOutput").ap()

    from contextlib import ExitStack
    with tile.TileContext(nc) as tc, ExitStack() as ctx:
        consts = ctx.enter_context(tc.tile_pool(name="consts", bufs=1))
        wpool = ctx.enter_context(tc.tile_pool(name="wpool", bufs=2))
        xb0_pool = ctx.enter_context(tc.tile_pool(name="xb0", bufs=4))
        xb_pool = ctx.enter_context(tc.tile_pool(name="xb", bufs=8))
        xnb_p = ctx.enter_context(tc.tile_pool(name="xnb", bufs=8))
        xna_p = ctx.enter_context(tc.tile_pool(name="xna", bufs=6))
        ha_p = ctx.enter_context(tc.tile_pool(name="ha", bufs=6))
        hb_pool = ctx.enter_context(tc.tile_pool(name="hb", bufs=2))
        sm = ctx.enter_context(tc.tile_pool(name="small", bufs=8))
        out_p = ctx.enter_context(tc.tile_pool(name="outp", bufs=2))
        psc = ctx.enter_context(tc.tile_pool(name="psc", bufs=4, space="PSUM"))
        psy = ctx.enter_context(tc.tile_pool(name="psy", bufs=2, space="PSUM"))

        fc_sb = consts.tile([128, 2, 2], BF16)
        nc.sync.dma_start(out=fc_sb, in_=fc_in)
        eps_sb = consts.tile([128, 1], F32)
        nc.vector.memset(eps_sb, EPS)

        # initial load: [BL, S, C] -> per-batch layout-B tiles (f32)
        xb_tiles = []
        for b in range(BL):
            t = xb0_pool.tile([128, NT, C], F32, tag="xB0")
            nc.sync.dma_start(out=t,
                              in_=x_in[b].rearrange("(t p) c -> p t c", p=128))
            xb_tiles.append(t)

        pools = (sm, xnb_p, xna_p, ha_p, psc, psy, xb_pool, hb_pool, out_p)
        g_tile = None
        for blk, k in enumerate(KS):
            w_sb = wpool.tile([128, 2, 2, k, 128], BF16, tag="w")
            nc.sync.dma_start(out=w_sb, in_=w_in[k])
            xb_tiles, g_tile = _build_block(
                nc, tc, pools, blk, k, xb_tiles, g_tile, w_sb, fc_sb,
                eps_sb, out_dram)

    nc.compile()
    return nc


def _get_nc():
    global _CACHED_NC
    if _CACHED_NC is None:
        _CACHED_NC = _build()
    return _CACHED_NC


def _prep_weights(inputs):
    """Host-side packing of conv / fc weights into the DRAM layouts."""
    arrs = {}
    for k in KS:
        W = np.asarray(inputs[f"W{k}"], np.float32)  # [co, ci, k]
        Wt = W.transpose(1, 0, 2)                    # [ci, co, k]
        Wt = Wt.reshape(2, 128, 2, 128, k)           # [ci_ch, ci_in, co_ch, co_in, k]
        Wt = Wt.transpose(1, 0, 2, 4, 3)             # [ci_in, ci_ch, co_ch, k, co_in]
        arrs[f"w{k}"] = np.ascontiguousarray(Wt).astype(ml_dtypes.bfloat16)
    fc = np.asarray(inputs["fc_w"], np.float32)[:, :, 0]  # [2, co=256]
    fcT = fc.T.reshape(2, 128, 2).transpose(1, 0, 2)      # [co_in, co_ch, cls]
    arrs["fc"] = np.ascontiguousarray(fcT).astype(ml_dtypes.bfloat16)
    return arrs


def _apply_cn_affine(inputs):
    """The kernel folds ChannelNorm's (w, b) away assuming w==1, b==0
    (true for this model's initialization). Verify on host."""
    for k in KS:
        w = np.asarray(inputs[f"cn{k}_w"], np.float32)
        bb = np.asarray(inputs[f"cn{k}_b"], np.float32)
        if not (np.allclose(w, 1.0, atol=1e-6) and
                np.allclose(bb, 0.0, atol=1e-6)):
            raise NotImplementedError(
                "kernel assumes channel-norm weight==1, bias==0")


def kernel(run_opts=None, **inputs):
    _apply_cn_affine(inputs)
    nc = _get_nc()
    warrs = _prep_weights(inputs)
    x = np.ascontiguousarray(np.asarray(inputs["inputs"], np.float32))
    in_maps = []
    for c in range(NCORES):
        m = {"x": np.ascontiguousarray(x[c * BL:(c + 1) * BL])}
        m.update(warrs)
        in_maps.append(m)
    res = run_bass_kernel_spmd(nc, in_maps, core_ids=list(range(NCORES)),
                               **(run_opts or {}))
    out = np.concatenate([r["out"] for r in res.results], axis=0)
    if run_opts:
        return out, res
    return out


# revision 5
# speedup vs baseline: 1.2186x; 1.1221x over previous
"""Trainium2 Bass/Tile kernel for a 5-block 1D-CNN stack
(ChannelNorm -> ReLU -> Conv1d(k=4..8) -> sigmoid gate -> re-pad).

Data-parallel over batch: 32 samples -> 4 per NeuronCore x 8 cores.

Per-core strategy (v3):
  * layout B [S(part), C(free)] for channel-norm stats (free-dim
    bn_stats) + fused normalize/relu (per-partition ACT scale/bias);
    layout A [C(part), S(free)] for the conv (PE matmuls over C_in).
  * ALL layout conversions use the DMA XBAR transpose (16-bit):
    out[p,t,q] = in[q, t*128+p] turns [128, T*128] into T transposed
    [128,128] blocks in one descriptor. B->A XBARs issue on the SP
    queue, A->B XBARs on the Activation queue (right after the psum->
    SBUF copies that produce their input, so they never stall it).
  * conv + gate matmuls in bf16 (1 PE cycle/row, fp32 PSUM accum);
    the PE runs only the conv (its ~205us roofline) + tiny gate GEMMs.
  * gate multiplier g = 1+score folds into the next block's norm
    coefficients (exact); only the last block materializes gated out.
  * scalar engine uses one act-table set {Relu, Copy, Ln, Exp}:
    rstd = exp(-0.5*ln(var')), sigmoid(z) = 1/(1+exp(-z)) via DVE
    reciprocal. get_activation_tables is shadowed so the table pass
    picks the one set that truly holds all four (no reloads).
  * software-pipelined emission: task (block i, sample s) is split
    into conv / gate+coef+norm halves, and the gate half of task t is
    emitted after the conv of task t+1, so every in-order engine queue
    sees work in the order it becomes ready (PE stays gapless).
"""

import functools

import numpy as np
import ml_dtypes

import concourse.bass as bass
import concourse.mybir as mybir
import concourse.tile as tile
from concourse import bacc
from concourse import hw_specs as _hw_specs
from concourse.bass_utils import run_bass_kernel_spmd

B, S, C = 32, 1024, 256
NCORES = 8
BL = B // NCORES  # batch per core
KS = [4, 5, 6, 7, 8]
PADS = {4: 1, 5: 2, 6: 2, 7: 3, 8: 3}
LS = {k: S + 2 * PADS[k] - k + 1 for k in KS}  # conv output lengths
EPS = 1e-5
UNB = float(C) / float(C - 1)  # unbiased-variance correction (ddof=1)
NT = S // 128  # 8 s-tiles of 128 per sample
NB = len(KS)
F32 = mybir.dt.float32
BF16 = mybir.dt.bfloat16
AF = mybir.ActivationFunctionType

# ---- activation-table pass steering -------------------------------------
# The table-load pass maps each function to the FIRST act_func_set that
# contains it, which scatters {Relu, Copy, Ln, Exp} over three sets and
# inserts a 1.28us table load per alternation.  One real set
# ("natural_log_exp_and_others") contains all four; hide them from every
# other set so the pass settles there.  Membership claims stay truthful,
# so the emitted set id loads the right table on hardware.
_ORIG_GAT = _hw_specs.get_activation_tables
_OUR_FUNCS = frozenset({AF.Relu, AF.Copy, AF.Ln, AF.Exp})
_KEEP_SET = "natural_log_exp_and_others"


@functools.cache
def _patched_gat(module_arch):
    tabs = _ORIG_GAT(module_arch)
    out = {}
    for name, funcs in tabs.items():
        fs = set(funcs)
        if name != _KEEP_SET:
            fs -= _OUR_FUNCS
        out[name] = fs
    return out


_hw_specs.get_activation_tables = _patched_gat
bacc.get_activation_tables = _patched_gat

_CACHED_NC = None


class _Pipe:
    """Per-build emission state shared by the pipeline stages."""

    def __init__(self, nc, pools, w_sb, fc_sb, eps_sb, out_dram):
        self.nc = nc
        (self.sm, self.xnb_p, self.xna_p, self.ha_p, self.psc, self.psy,
         self.xb_pool, self.hb_pool, self.out_p) = pools
        self.w_sb = w_sb
        self.fc_sb = fc_sb
        self.eps_sb = eps_sb
        self.out_dram = out_dram

    # ---- half A: stats + norm coefficients + normalize/relu + B->A ----
    def half_a(self, i, s, xb_t, g_prev):
        nc = self.nc
        sm = self.sm
        bn6 = sm.tile([128, NT, 6], F32, tag="bn6")
        for st in range(NT):
            nc.vector.bn_stats(out=bn6[:, st, :], in_=xb_t[:, st, :])
        mv = sm.tile([128, NT, 2], F32, tag="mv")
        for st in range(NT):
            nc.vector.bn_aggr(out=mv[:, st, :], in_=bn6[:, st, :])
        mean = mv[:, :, 0:1].rearrange("p t o -> p (t o)")  # [128, NT]
        var = mv[:, :, 1:2].rearrange("p t o -> p (t o)")

        g2v = sm.tile([128, NT], F32, tag="g2v")
        if g_prev is None:
            # single DVE reader collapses the 8 bn_aggr deps into one
            # same-engine chain (walrus caps sync waits per instruction)
            nc.vector.tensor_copy(g2v, var)
        else:
            nc.vector.tensor_mul(g2v, g_prev, g_prev)
            nc.vector.tensor_mul(g2v, g2v, var)
        # rr = 1/sqrt(g^2*var*UNB + eps) = exp(-0.5*ln(...)) -- Ln and Exp
        # share the one loaded act-table set.
        lnv = sm.tile([128, NT], F32, tag="lnv")
        nc.scalar.activation(out=lnv, in_=g2v, func=AF.Ln,
                             bias=self.eps_sb, scale=UNB)
        rr = sm.tile([128, NT], F32, tag="rr")
        nc.scalar.activation(out=rr, in_=lnv, func=AF.Exp, scale=-0.5)
        scale_c = sm.tile([128, NT], F32, tag="scale_c")
        if g_prev is None:
            nc.vector.tensor_copy(scale_c, rr)
        else:
            nc.vector.tensor_mul(scale_c, rr, g_prev)
        bias_c = sm.tile([128, NT], F32, tag="bias_c")
        # bias = -(mean * scale)
        nc.vector.scalar_tensor_tensor(out=bias_c, in0=mean, scalar=-1.0,
                                       in1=scale_c,
                                       op0=mybir.AluOpType.mult,
                                       op1=mybir.AluOpType.mult)

        xna_b = self.xna_p.tile([128, 2, S], BF16, tag="xnA")  # [ci_ch, s]
        for st in range(NT):
            xnb_t = self.xnb_p.tile([128, C], BF16, tag="xnB")
            nc.scalar.activation(out=xnb_t, in_=xb_t[:, st, :],
                                 func=AF.Relu,
                                 scale=scale_c[:, st:st + 1],
                                 bias=bias_c[:, st:st + 1])
            # XBAR: out[ci, ch, s] = xnb[s, ch*128+ci]
            nc.sync.dma_start(out=xna_b[:, :, st * 128:(st + 1) * 128],
                              in_=xnb_t, transpose=True)
        return xna_b

    # ---- conv + psum->SBUF + re-pad + A->B XBAR ----
    def conv(self, i, s, xna_b):
        nc = self.nc
        k = KS[i]
        p = PADS[k]
        L = LS[k]
        last = (i == NB - 1)
        w_sb = self.w_sb[k]

        ha_b = self.ha_p.tile([128, 2, S], BF16, tag="hA")  # [co_ch, s]
        for t in range(2):
            for co in range(2):
                pc = self.psc.tile([128, 512], F32, tag="psc")
                # tap (ci=0, dk=p) first: full 512 width so start=True
                # initializes the whole psum tile.
                taps = [(0, p)] + [(ci, dk) for ci in range(2)
                                   for dk in range(k) if (ci, dk) != (0, p)]
                for idx, (ci, dk) in enumerate(taps):
                    lo = t * 512 + dk - p  # src col of psum col 0
                    a = max(0, -lo)
                    bb = 512 - max(0, lo + 512 - S)
                    nc.tensor.matmul(pc[:, a:bb],
                                     w_sb[:, ci, co, dk, :],
                                     xna_b[:, ci, lo + a:lo + bb],
                                     start=(idx == 0),
                                     stop=(idx == len(taps) - 1))
                nc.scalar.copy(out=ha_b[:, co, t * 512:(t + 1) * 512],
                               in_=pc)
        if L < S:  # zero the re-pad tail
            for co in range(2):
                nc.vector.memset(ha_b[:, co, L:S], 0.0)

        # A->B XBAR back to layout B for the next block's stats/normalize.
        if last:
            dst_t = self.hb_pool.tile([128, NT, C], BF16, tag="hb")
        else:
            dst_t = self.xb_pool.tile([128, NT, C], BF16, tag="xB")
        for co in range(2):
            nc.sync.dma_start(out=dst_t[:, :, co * 128:(co + 1) * 128],
                              in_=ha_b[:, co, :], transpose=True)
        return ha_b, dst_t

    # ---- gate matmuls + gate math (+ final gated output for last blk) ----
    def gate(self, i, s, ha_b, dst_t):
        nc = self.nc
        sm = self.sm
        last = (i == NB - 1)
        ya2 = sm.tile([128, NT, 2], F32, tag="ya2")
        for st in range(NT):
            py = self.psy.tile([128, 2], F32, tag="psy")
            for co in range(2):
                nc.tensor.matmul(py, ha_b[:, co, st * 128:(st + 1) * 128],
                                 self.fc_sb[:, co, :],
                                 start=(co == 0), stop=(co == 1))
            nc.vector.tensor_copy(out=ya2[:, st, :], in_=py)
        ya = ya2[:, :, 0:1].rearrange("p t o -> p (t o)")  # [128, NT]
        yb = ya2[:, :, 1:2].rearrange("p t o -> p (t o)")
        smax = sm.tile([128, NT], F32, tag="smax")
        nc.vector.tensor_max(smax, ya, yb)
        ssum = sm.tile([128, NT], F32, tag="ssum")
        nc.vector.tensor_add(ssum, ya, yb)
        pre = sm.tile([128, NT], F32, tag="pre")
        # pre = 0.2*max + (ya+yb);  score = sigmoid(0.5*pre)
        nc.vector.scalar_tensor_tensor(out=pre, in0=smax, scalar=0.2,
                                       in1=ssum,
                                       op0=mybir.AluOpType.mult,
                                       op1=mybir.AluOpType.add)
        # sigmoid via exp + DVE reciprocal (single act-table set)
        ex = sm.tile([128, NT], F32, tag="ex")
        nc.scalar.activation(out=ex, in_=pre, func=AF.Exp, scale=-0.5)
        den = sm.tile([128, NT], F32, tag="den")
        nc.vector.tensor_scalar_add(den, ex, 1.0)
        g_t = sm.tile([128, NT], F32, tag="g")
        nc.vector.reciprocal(g_t, den)
        nc.vector.tensor_scalar_add(g_t, g_t, 1.0)

        if last:
            out_t = self.out_p.tile([128, NT, C], F32, tag="osb")
            for st in range(NT):
                nc.vector.tensor_scalar_mul(out=out_t[:, st, :],
                                            in0=dst_t[:, st, :],
                                            scalar1=g_t[:, st:st + 1])
            dst = self.out_dram[s].rearrange("(t p) c -> p t c", p=128)
            nc.sync.dma_start(out=dst, in_=out_t)
        return g_t


def _build():
    nc = bacc.Bacc("TRN2", target_bir_lowering=False, debug=False,
                   num_devices=NCORES)
    x_in = nc.dram_tensor("x", [BL, S, C], F32, kind="ExternalInput").ap()
    w_in = {k: nc.dram_tensor(f"w{k}", [128, 2, 2, k, 128], BF16,
                              kind="ExternalInput").ap() for k in KS}
    fc_in = nc.dram_tensor("fc", [128, 2, 2], BF16,
                           kind="ExternalInput").ap()
    out_dram = nc.dram_tensor("out", [BL, S, C], F32,
                              kind="ExternalOutput").ap()

    from contextlib import ExitStack
    with tile.TileContext(nc) as tc, ExitStack() as ctx:
        consts = ctx.enter_context(tc.tile_pool(name="consts", bufs=1))
        xb0_pool = ctx.enter_context(tc.tile_pool(name="xb0", bufs=4))
        xb_pool = ctx.enter_context(tc.tile_pool(name="xb", bufs=8))
        xnb_p = ctx.enter_context(tc.tile_pool(name="xnb", bufs=8))
        xna_p = ctx.enter_context(tc.tile_pool(name="xna", bufs=6))
        ha_p = ctx.enter_context(tc.tile_pool(name="ha", bufs=6))
        hb_pool = ctx.enter_context(tc.tile_pool(name="hb", bufs=3))
        sm = ctx.enter_context(tc.tile_pool(name="small", bufs=8))
        out_p = ctx.enter_context(tc.tile_pool(name="outp", bufs=3))
        psc = ctx.enter_context(tc.tile_pool(name="psc", bufs=4, space="PSUM"))
        psy = ctx.enter_context(tc.tile_pool(name="psy", bufs=2, space="PSUM"))

        fc_sb = consts.tile([128, 2, 2], BF16)
        nc.sync.dma_start(out=fc_sb, in_=fc_in)
        eps_sb = consts.tile([128, 1], F32)
        nc.vector.memset(eps_sb, EPS)

        # initial load: [BL, S, C] -> per-batch layout-B tiles (f32)
        xb0 = []
        for s in range(BL):
            t = xb0_pool.tile([128, NT, C], F32, tag="xB0")
            nc.sync.dma_start(out=t,
                              in_=x_in[s].rearrange("(t p) c -> p t c", p=128))
            xb0.append(t)
        # all conv weights up front (30KB/partition total in bf16)
        w_sb = {}
        for k in KS:
            w_sb[k] = consts.tile([128, 2, 2, k, 128], BF16, tag=f"w{k}",
                                  name=f"w{k}_sb")
            nc.sync.dma_start(out=w_sb[k], in_=w_in[k])

        pools = (sm, xnb_p, xna_p, ha_p, psc, psy, xb_pool, hb_pool, out_p)
        pipe = _Pipe(nc, pools, w_sb, fc_sb, eps_sb, out_dram)

        # software pipeline over tasks (block i, sample s):
        #   conv(t) ; then gate(t-1) ; then half_a(next block of t-1's
        # sample) -- so gate GEMMs never make the PE wait on the psum->
        # SBUF copies, and each engine queue sees work in ready-order.
        xna = {}
        for s in range(BL):
            xna[s] = pipe.half_a(0, s, xb0[s], None)
        pend = None
        for i in range(NB):
            for s in range(BL):
                ha_b, dst_t = pipe.conv(i, s, xna[s])
                if pend is not None:
                    ip, sp, ha_p_, dst_p = pend
                    g_t = pipe.gate(ip, sp, ha_p_, dst_p)
                    if ip + 1 < NB:
                        xna[sp] = pipe.half_a(ip + 1, sp, dst_p, g_t)
                pend = (i, s, ha_b, dst_t)
        pipe.gate(*pend[0:2], pend[2], pend[3])

    nc.compile()
    return nc


def _get_nc():
    global _CACHED_NC
    if _CACHED_NC is None:
        _CACHED_NC = _build()
    return _CACHED_NC


def _prep_weights(inputs):
    """Host-side packing of conv / fc weights into the DRAM layouts."""
    arrs = {}
    for k in KS:
        W = np.asarray(inputs[f"W{k}"], np.float32)  # [co, ci, k]
        Wt = W.transpose(1, 0, 2)                    # [ci, co, k]
        Wt = Wt.reshape(2, 128, 2, 128, k)           # [ci_ch, ci_in, co_ch, co_in, k]
        Wt = Wt.transpose(1, 0, 2, 4, 3)             # [ci_in, ci_ch, co_ch, k, co_in]
        arrs[f"w{k}"] = np.ascontiguousarray(Wt).astype(ml_dtypes.bfloat16)
    fc = np.asarray(inputs["fc_w"], np.float32)[:, :, 0]  # [2, co=256]
    fcT = fc.T.reshape(2, 128, 2).transpose(1, 0, 2)      # [co_in, co_ch, cls]
    arrs["fc"] = np.ascontiguousarray(fcT).astype(ml_dtypes.bfloat16)
    return arrs


def _apply_cn_affine(inputs):
    """The kernel folds ChannelNorm's (w, b) away assuming w==1, b==0
    (true for this model's initialization). Verify on host."""
    for k in KS:
        w = np.asarray(inputs[f"cn{k}_w"], np.float32)
        bb = np.asarray(inputs[f"cn{k}_b"], np.float32)
        if not (np.allclose(w, 1.0, atol=1e-6) and
                np.allclose(bb, 0.0, atol=1e-6)):
            raise NotImplementedError(
                "kernel assumes channel-norm weight==1, bias==0")


def kernel(run_opts=None, **inputs):
    _apply_cn_affine(inputs)
    nc = _get_nc()
    warrs = _prep_weights(inputs)
    x = np.ascontiguousarray(np.asarray(inputs["inputs"], np.float32))
    in_maps = []
    for c in range(NCORES):
        m = {"x": np.ascontiguousarray(x[c * BL:(c + 1) * BL])}
        m.update(warrs)
        in_maps.append(m)
    res = run_bass_kernel_spmd(nc, in_maps, core_ids=list(range(NCORES)),
                               **(run_opts or {}))
    out = np.concatenate([r["out"] for r in res.results], axis=0)
    if run_opts:
        return out, res
    return out
